# revision 1
# baseline (speedup 1.0000x reference)
"""Trainium2 Bass kernel for local-correlation + masked top-256 (sparse_attention).

Contract: kernel(**inputs) takes FULL unsharded inputs (pre, curr, mask, mode)
and returns the full output tuple (f, b), each [4, 256, 128, 128] f32.

Sharding: pure data parallel over (batch, H-half) -> 8 cores.
Per core:
  - L2-normalize pre/curr over C (sumsq via ones-matmul, invn = Exp(-0.5*Ln(ss))).
  - For each of 64 output rows h: 17 Gram matmuls cur_n[:,h,:]^T @ pre_n[:,h+dy,:]
    -> [128w, 144w'] in PSUM; copied into a staging tile, round-tripped through
    DRAM with write-row-stride 2449 and read-partition-stride 2450 so the
    diagonal band co[w, dy*17+dx] = g[dy][w, w+dx] comes back as one DMA.
  - xf = co*m (mask unfolded on host), xb = co - xf.
  - top-256 sorted desc via 32 rounds of vector max8 + match_replace(-3.0).
  - PE transpose [w,k] -> [k,w], DMA to [256, 64, 128] outputs.
"""

import numpy as np

K = 8
KW = 17
D = KW * KW            # 289
TOPK = 256
B, C, H, W = 4, 256, 128, 128
N_CORES = 8
HSLICE = H // 2        # 64 rows per core
WP = W + 2 * K         # 144
HP = HSLICE + 2 * K    # 80
NROUND = TOPK // 8     # 32
PRE_COLS = HP * WP     # 11520
CUR_COLS = HSLICE * W  # 8192
STG = KW * WP          # 2448
SCR_FLAT = 128 * (STG + 2)   # 313600 : divisible by 2450; first 128*2449 used for write view

_CACHED = {"nc": None}


def _build_nc():
    import concourse.bacc as bacc
    import concourse.tile as tile
    import concourse.mybir as mybir

    f32 = mybir.dt.float32
    AF = mybir.ActivationFunctionType
    ALU = mybir.AluOpType

    nc = bacc.Bacc("TRN2", target_bir_lowering=False, debug=False,
                   enable_asserts=False, num_devices=N_CORES)

    pre_d = nc.dram_tensor("pre_pad", [C, HP, WP], f32, kind="ExternalInput").ap()
    cur_d = nc.dram_tensor("curr", [C, HSLICE, W], f32, kind="ExternalInput").ap()
    m_d = nc.dram_tensor("m_unf", [HSLICE, W, D], f32, kind="ExternalInput").ap()
    f_d = nc.dram_tensor("f_out", [TOPK, HSLICE, W], f32, kind="ExternalOutput").ap()
    b_d = nc.dram_tensor("b_out", [TOPK, HSLICE, W], f32, kind="ExternalOutput").ap()
    scr = [nc.dram_tensor(f"scr{i}", [SCR_FLAT], f32, kind="Internal").ap()
           for i in range(2)]

    ident_d = nc.inline_tensor(np.eye(128, dtype=np.float32), name="ident")
    ones_col_d = nc.inline_tensor(np.ones((128, 1), np.float32), name="ones_col")
    ones_row_d = nc.inline_tensor(np.ones((1, 128), np.float32), name="ones_row")

    with tile.TileContext(nc) as tc:
        with tc.tile_pool(name="persist", bufs=1) as pp:
            pre_n = [pp.tile([128, PRE_COLS], f32, tag=f"pre{c}", name=f"pre{c}") for c in range(2)]
            cur_n = [pp.tile([128, CUR_COLS], f32, tag=f"cur{c}", name=f"cur{c}") for c in range(2)]
            ident = pp.tile([128, 128], f32, tag="ident", name="identt")
            ones_col = pp.tile([128, 1], f32, tag="onc", name="onc")
            ones_row = pp.tile([1, 128], f32, tag="onr", name="onr")
            nc.sync.dma_start(ident[:, :], ident_d.ap())
            nc.sync.dma_start(ones_col[:, :], ones_col_d.ap())
            nc.sync.dma_start(ones_row[:, :], ones_row_d.ap())
            for c in range(2):
                nc.sync.dma_start(
                    pre_n[c][:, :],
                    pre_d[c * 128:(c + 1) * 128, :, :].rearrange("c hh ww -> c (hh ww)"))
                nc.sync.dma_start(
                    cur_n[c][:, :],
                    cur_d[c * 128:(c + 1) * 128, :, :].rearrange("c hh ww -> c (hh ww)"))

            # ---- Stage A: L2 normalization over C (in place) ----
            with (tc.tile_pool(name="sqp", bufs=2) as sqp,
                  tc.tile_pool(name="strip", bufs=2) as stp,
                  tc.tile_pool(name="ssp", bufs=2, space="PSUM") as ssp,
                  tc.tile_pool(name="bcp", bufs=2, space="PSUM") as bcp):
                for tiles, ncols, cw in ((pre_n, PRE_COLS, 480), (cur_n, CUR_COLS, 512)):
                    for j in range(ncols // cw):
                        cs = slice(j * cw, (j + 1) * cw)
                        ss = ssp.tile([1, cw], f32, tag="ss", name="ss")
                        for c in range(2):
                            sq = sqp.tile([128, cw], f32, tag="sq", name="sq")
                            nc.scalar.activation(sq[:, :], tiles[c][:, cs], AF.Square)
                            nc.tensor.matmul(ss[:, :], ones_col[:, :], sq[:, :],
                                             start=(c == 0), stop=(c == 1))
                        lns = stp.tile([1, cw], f32, tag="lns", name="lns")
                        nc.scalar.activation(lns[:, :], ss[:, :], AF.Ln)
                        bc = bcp.tile([128, cw], f32, tag="bc", name="bc")
                        nc.tensor.matmul(bc[:, :], ones_row[:, :], lns[:, :],
                                         start=True, stop=True)
                        inv = sqp.tile([128, cw], f32, tag="inv", name="inv")
                        nc.scalar.activation(inv[:, :], bc[:, :], AF.Exp, scale=-0.5)
                        for c in range(2):
                            nc.gpsimd.tensor_tensor(
                                out=tiles[c][:, cs], in0=tiles[c][:, cs],
                                in1=inv[:, :], op=ALU.mult)

            # ---- Stage B: per-row gram, shear, mask, top-k extraction ----
            with (tc.tile_pool(name="stage", bufs=2) as stgp,
                  tc.tile_pool(name="cop", bufs=2) as cop,
                  tc.tile_pool(name="xp", bufs=4) as xp,
                  tc.tile_pool(name="fbp", bufs=4) as fbp,
                  tc.tile_pool(name="trp", bufs=2) as trp,
                  tc.tile_pool(name="gp", bufs=4, space="PSUM") as gp,
                  tc.tile_pool(name="tp", bufs=4, space="PSUM") as tp):
                for h in range(HSLICE):
                    stage = stgp.tile([128, STG], f32, tag="stage", name="stage")
                    for dy in range(KW):
                        g = gp.tile([128, WP], f32, tag="g", name="g")
                        for c in range(2):
                            nc.tensor.matmul(
                                g[:, :],
                                cur_n[c][:, h * W:(h + 1) * W],
                                pre_n[c][:, (h + dy) * WP:(h + dy + 1) * WP],
                                start=(c == 0), stop=(c == 1))
                        nc.scalar.activation(stage[:, dy * WP:(dy + 1) * WP],
                                             g[:, :], AF.Copy)
                    sc = scr[h % 2]
                    wview = sc[0:128 * (STG + 1)].rearrange("(p r) -> p r", r=STG + 1)
                    nc.sync.dma_start(wview[:, 0:STG], stage[:, :])
                    co = cop.tile([128, D], f32, tag="co", name="co")
                    rview = sc[:].rearrange("(p r) -> p r", r=STG + 2)
                    rview = rview[:, 0:STG].rearrange("p (a b) -> p a b", b=WP)
                    nc.sync.dma_start(co[:, :], rview[:, :, 0:KW])
                    m = cop.tile([128, D], f32, tag="m", name="m")
                    nc.sync.dma_start(m[:, :], m_d[h, :, :])
                    xf = xp.tile([128, D], f32, tag="xf", name="xf")
                    xb = xp.tile([128, D], f32, tag="xb", name="xb")
                    nc.gpsimd.tensor_tensor(out=xf[:, :], in0=co[:, :], in1=m[:, :],
                                            op=ALU.mult)
                    nc.gpsimd.tensor_tensor(out=xb[:, :], in0=co[:, :], in1=xf[:, :],
                                            op=ALU.subtract)
                    for x, out_d in ((xf, f_d), (xb, b_d)):
                        ft = fbp.tile([128, TOPK], f32, tag="ft", name="ft")
                        for r in range(NROUND):
                            nc.vector.max(ft[:, r * 8:(r + 1) * 8], x[:, :])
                            if r + 1 < NROUND:
                                nc.vector.match_replace(
                                    x[:, :], ft[:, r * 8:(r + 1) * 8], x[:, :],
                                    imm_value=-3.0)
                        tr = trp.tile([128, TOPK], f32, tag="tr", name="tr")
                        for half in range(2):
                            pt = tp.tile([128, 128], f32, tag="pt", name="pt")
                            nc.tensor.transpose(
                                pt[:, :], ft[:, half * 128:(half + 1) * 128],
                                ident[:, :])
                            nc.scalar.activation(
                                tr[:, half * 128:(half + 1) * 128], pt[:, :], AF.Copy)
                        oview = out_d.rearrange("(cc p) hh ww -> p cc hh ww", cc=2)
                        nc.sync.dma_start(
                            oview[:, :, h, :],
                            tr[:, :].rearrange("p (cc ww) -> p cc ww", ww=W))
    nc.compile()
    return nc


def _host_prep(pre, curr, mask):
    pre_pad = np.pad(pre, ((0, 0), (0, 0), (K, K), (K, K)), mode="reflect")
    mask_pad = np.pad(mask, ((0, 0), (0, 0), (K, K), (K, K)))
    # unfold mask: m[h, w, dy*17+dx] = mask_pad[h+dy, w+dx]
    ins = []
    for k in range(N_CORES):
        b, hh = k // 2, k % 2
        h0 = hh * HSLICE
        mp = mask_pad[b, 0, h0:h0 + HP, :]
        s0, s1 = mp.strides
        m_unf = np.lib.stride_tricks.as_strided(
            mp, (HSLICE, KW, W, KW), (s0, s0, s1, s1))
        m_unf = np.ascontiguousarray(
            m_unf.transpose(0, 2, 1, 3).reshape(HSLICE, W, D))
        ins.append({
            "pre_pad": np.ascontiguousarray(pre_pad[b, :, h0:h0 + HP, :]),
            "curr": np.ascontiguousarray(curr[b, :, h0:h0 + HSLICE, :]),
            "m_unf": m_unf,
        })
    return ins


def kernel(pre, curr, mask, mode):
    from concourse.bass_utils import run_bass_kernel_spmd

    pre = np.asarray(pre, dtype=np.float32)
    curr = np.asarray(curr, dtype=np.float32)
    mask = np.asarray(mask, dtype=np.float32)
    assert int(np.asarray(mode)) == 0

    if _CACHED["nc"] is None:
        _CACHED["nc"] = _build_nc()
    nc = _CACHED["nc"]

    in_maps = _host_prep(pre, curr, mask)
    res = run_bass_kernel_spmd(nc, in_maps, core_ids=list(range(N_CORES)))
    f = np.zeros((B, TOPK, H, W), np.float32)
    bo = np.zeros((B, TOPK, H, W), np.float32)
    for k in range(N_CORES):
        bb, hh = k // 2, k % 2
        f[bb, :, hh * HSLICE:(hh + 1) * HSLICE, :] = res.results[k]["f_out"]
        bo[bb, :, hh * HSLICE:(hh + 1) * HSLICE, :] = res.results[k]["b_out"]
    return (f, bo)



# revision 3
# speedup vs baseline: 1.5897x; 1.5897x over previous
"""Trainium2 Bass kernel for local-correlation + masked top-256 (sparse_attention).

Contract: kernel(**inputs) takes FULL unsharded inputs (pre, curr, mask, mode)
and returns the full output tuple (f, b), each [4, 256, 128, 128] f32.

Sharding: pure data parallel over (batch, H-half) -> 8 cores.

Algorithm per core (fp16 data path; rel-err budget 2e-2, achieved ~2e-3):
  - L2-normalize pre/curr over C (sumsq via ones-matmul in fp16 -> PSUM f32,
    inv = Exp(-0.5*Ln(ss)) with Ln/Exp in f32, scale applied in fp16).
  - Per output row h: 17 Gram matmuls (fp16, 1 cyc/col) -> PSUM; copied fp16
    into a staging tile; DRAM round-trip with write-row-stride 2449 /
    read-partition-stride 2450 extracts the diagonal band co[w, dy*17+dx].
  - Mask-compaction via gpsimd local_scatter with host-computed per-pixel
    rank indices: xf-candidates (width WF = max n1) and xb-candidates
    (width WB = max n0) hold only the nonzero entries of co*m / co*(1-m);
    unwritten slots are 0 which sort exactly where the true zeros belong.
  - Sorted top-k by max8+match_replace rounds, but only RF/RB rounds
    (RF ~ (max n1)/8 + margin instead of 32): the mask-zeros block of each
    output is NOT extracted by the sorter.  Placement into the final
    [128, 256] output is a second local_scatter with dest = j + (289-W)*[v<0]
    (a compile-time constant offset; dst pre-zeroed by the instruction
    provides the zero block; overflow slots land in a junk region [256,512)).
  - fp16 -> f32 conversion, PE transpose [w,k] -> [k,w], DMA out.

Compile-time parameters (WF, WB, RF, RB) derive from the actual input mask;
the compiled module is cached per parameter tuple.
"""

import numpy as np

K = 8
KW = 17
D = KW * KW            # 289
TOPK = 256
B, C, H, W = 4, 256, 128, 128
N_CORES = 8
HSLICE = H // 2        # 64 rows per core
WP = W + 2 * K         # 144
HP = HSLICE + 2 * K    # 80
PRE_COLS = HP * WP     # 11520
CUR_COLS = HSLICE * W  # 8192
STG = KW * WP          # 2448
SCR_FLAT = 128 * (STG + 2)   # divisible by 2450; first 128*2449 used for write view
NIC = D + 1            # 290: compaction scatter num_idxs (even)

_CACHED = {}


def _build_nc(WF, WB, RF, RB):
    import concourse.bacc as bacc
    import concourse.tile as tile
    import concourse.mybir as mybir

    f32 = mybir.dt.float32
    f16 = mybir.dt.float16
    i16 = mybir.dt.int16
    AF = mybir.ActivationFunctionType
    ALU = mybir.AluOpType
    NF8 = RF * 8
    NB8 = RB * 8

    nc = bacc.Bacc("TRN2", target_bir_lowering=False, debug=False,
                   enable_asserts=False, num_devices=N_CORES)

    pre_d = nc.dram_tensor("pre_pad", [C, HP, WP], f16, kind="ExternalInput").ap()
    cur_d = nc.dram_tensor("curr", [C, HSLICE, W], f16, kind="ExternalInput").ap()
    idxf_d = nc.dram_tensor("idx_f", [HSLICE, 128, NIC], i16, kind="ExternalInput").ap()
    idxb_d = nc.dram_tensor("idx_b", [HSLICE, 128, NIC], i16, kind="ExternalInput").ap()
    f_d = nc.dram_tensor("f_out", [TOPK, HSLICE, W], f32, kind="ExternalOutput").ap()
    b_d = nc.dram_tensor("b_out", [TOPK, HSLICE, W], f32, kind="ExternalOutput").ap()
    scr = [nc.dram_tensor(f"scr{i}", [SCR_FLAT], f16, kind="Internal").ap()
           for i in range(2)]

    ident_d = nc.inline_tensor(np.eye(128, dtype=np.float32), name="ident")
    ones_col_d = nc.inline_tensor(np.ones((128, 1), np.float16), name="ones_col")
    ones_row_d = nc.inline_tensor(np.ones((1, 128), np.float16), name="ones_row")
    iota_d = nc.inline_tensor(
        np.tile(np.arange(256, dtype=np.float16), (128, 1)), name="iota")

    # gram grouping: 17 dys in PSUM tiles of 3+3+3+3+3+2
    GRP = [(0, 3), (3, 3), (6, 3), (9, 3), (12, 3), (15, 2)]

    with tile.TileContext(nc) as tc:
        with tc.tile_pool(name="persist", bufs=1) as pp:
            pre_n = [pp.tile([128, PRE_COLS], f16, tag=f"pre{c}", name=f"pre{c}") for c in range(2)]
            cur_n = [pp.tile([128, CUR_COLS], f16, tag=f"cur{c}", name=f"cur{c}") for c in range(2)]
            ident = pp.tile([128, 128], f32, tag="ident", name="identt")
            ones_col = pp.tile([128, 1], f16, tag="onc", name="onc")
            ones_row = pp.tile([1, 128], f16, tag="onr", name="onr")
            iota = pp.tile([128, 256], f16, tag="iota", name="iota")
            nc.sync.dma_start(ident[:, :], ident_d.ap())
            nc.sync.dma_start(ones_col[:, :], ones_col_d.ap())
            nc.sync.dma_start(ones_row[:, :], ones_row_d.ap())
            nc.sync.dma_start(iota[:, :], iota_d.ap())
            for c in range(2):
                nc.sync.dma_start(
                    pre_n[c][:, :],
                    pre_d[c * 128:(c + 1) * 128, :, :].rearrange("c hh ww -> c (hh ww)"))
                nc.sync.dma_start(
                    cur_n[c][:, :],
                    cur_d[c * 128:(c + 1) * 128, :, :].rearrange("c hh ww -> c (hh ww)"))

            # ---- Stage A: L2 normalization over C (in place, fp16) ----
            with (tc.tile_pool(name="sqp", bufs=2) as sqp,
                  tc.tile_pool(name="strip", bufs=2) as stp,
                  tc.tile_pool(name="ssp", bufs=2, space="PSUM") as ssp,
                  tc.tile_pool(name="bcp", bufs=2, space="PSUM") as bcp):
                for tiles, ncols, cw in ((pre_n, PRE_COLS, 480), (cur_n, CUR_COLS, 512)):
                    for j in range(ncols // cw):
                        cs = slice(j * cw, (j + 1) * cw)
                        ss = ssp.tile([1, cw], f32, tag="ss", name="ss")
                        for c in range(2):
                            sq = sqp.tile([128, cw], f16, tag="sq", name="sq")
                            nc.scalar.activation(sq[:, :], tiles[c][:, cs], AF.Square)
                            nc.tensor.matmul(ss[:, :], ones_col[:, :], sq[:, :],
                                             start=(c == 0), stop=(c == 1))
                        lns = stp.tile([1, cw], f32, tag="lns", name="lns")
                        nc.scalar.activation(lns[:, :], ss[:, :], AF.Ln)
                        inv1 = stp.tile([1, cw], f16, tag="inv1", name="inv1")
                        nc.scalar.activation(inv1[:, :], lns[:, :], AF.Exp, scale=-0.5)
                        bc = bcp.tile([128, cw], f32, tag="bc", name="bc")
                        nc.tensor.matmul(bc[:, :], ones_row[:, :], inv1[:, :],
                                         start=True, stop=True)
                        inv = sqp.tile([128, cw], f16, tag="inv", name="inv")
                        nc.scalar.activation(inv[:, :], bc[:, :], AF.Copy)
                        for c in range(2):
                            nc.gpsimd.tensor_tensor(
                                out=tiles[c][:, cs], in0=tiles[c][:, cs],
                                in1=inv[:, :], op=ALU.mult)

            # ---- Stage B: per-row gram, shear, compact, sort, place ----
            with (tc.tile_pool(name="stage", bufs=2) as stgp,
                  tc.tile_pool(name="cop", bufs=2) as cop,
                  tc.tile_pool(name="idxp", bufs=2) as idxp,
                  tc.tile_pool(name="xcp", bufs=2) as xcp,
                  tc.tile_pool(name="fbp", bufs=2) as fbp,
                  tc.tile_pool(name="dstp", bufs=2) as dstp,
                  tc.tile_pool(name="fop", bufs=2) as fop,
                  tc.tile_pool(name="cvp", bufs=2) as cvp,
                  tc.tile_pool(name="trp", bufs=2) as trp,
                  tc.tile_pool(name="gp", bufs=1, space="PSUM") as gp,
                  tc.tile_pool(name="tp", bufs=1, space="PSUM") as tp):
                for h in range(HSLICE):
                    stage = stgp.tile([128, STG], f16, tag="stage", name="stage")
                    for gi, (dy0, ndy) in enumerate(GRP):
                        g = gp.tile([128, ndy * WP], f32, tag=f"g{gi}", name=f"g{gi}")
                        for li in range(ndy):
                            dy = dy0 + li
                            for c in range(2):
                                nc.tensor.matmul(
                                    g[:, li * WP:(li + 1) * WP],
                                    cur_n[c][:, h * W:(h + 1) * W],
                                    pre_n[c][:, (h + dy) * WP:(h + dy + 1) * WP],
                                    start=(c == 0), stop=(c == 1))
                        nc.scalar.activation(
                            stage[:, dy0 * WP:(dy0 + ndy) * WP], g[:, :], AF.Copy)
                    sc = scr[h % 2]
                    wview = sc[0:128 * (STG + 1)].rearrange("(p r) -> p r", r=STG + 1)
                    nc.sync.dma_start(wview[:, 0:STG], stage[:, :])
                    co = cop.tile([128, NIC], f16, tag="co", name="co")
                    rview = sc[:].rearrange("(p r) -> p r", r=STG + 2)
                    rview = rview[:, 0:STG].rearrange("p (a b) -> p a b", b=WP)
                    nc.sync.dma_start(co[:, 0:D], rview[:, :, 0:KW])
                    nc.gpsimd.memset(co[:, D:NIC], -3.0)

                    idxf = idxp.tile([128, NIC], i16, tag="idxf", name="idxf")
                    idxb = idxp.tile([128, NIC], i16, tag="idxb", name="idxb")
                    nc.sync.dma_start(idxf[:, :], idxf_d[h, :, :])
                    nc.sync.dma_start(idxb[:, :], idxb_d[h, :, :])

                    for (idxt, WX, NX8, out_d, tag) in (
                            (idxf, WF, NF8, f_d, "f"),
                            (idxb, WB, NB8, b_d, "b")):
                        xc = xcp.tile([128, WX], f16, tag=f"xc{tag}", name=f"xc{tag}")
                        nc.gpsimd.local_scatter(
                            xc[:, :], co[:, :], idxt[:, :],
                            channels=128, num_elems=WX, num_idxs=NIC)
                        st = fbp.tile([128, 256], f16, tag=f"st{tag}", name=f"st{tag}")
                        for r in range(NX8 // 8):
                            nc.vector.max(st[:, r * 8:(r + 1) * 8], xc[:, :])
                            if (r + 1) * 8 < NX8:
                                nc.vector.match_replace(
                                    xc[:, :], st[:, r * 8:(r + 1) * 8], xc[:, :],
                                    imm_value=-3.0)
                        # dest = j + (289 - WX) * [v < 0]; junk region >= 256
                        ng = dstp.tile([128, 256], f16, tag=f"ng{tag}", name=f"ng{tag}")
                        nc.gpsimd.tensor_scalar(
                            out=ng[:, 0:NX8], in0=st[:, 0:NX8],
                            scalar1=0.0, scalar2=float(D - WX),
                            op0=ALU.is_lt, op1=ALU.mult)
                        nc.gpsimd.tensor_tensor(
                            out=ng[:, 0:NX8], in0=ng[:, 0:NX8],
                            in1=iota[:, 0:NX8], op=ALU.add)
                        di = dstp.tile([128, 256], i16, tag=f"di{tag}", name=f"di{tag}")
                        nc.scalar.activation(di[:, 0:NX8], ng[:, 0:NX8], AF.Copy)
                        fo = fop.tile([128, 512], f16, tag=f"fo{tag}", name=f"fo{tag}")
                        nc.gpsimd.local_scatter(
                            fo[:, :], st[:, 0:NX8], di[:, 0:NX8],
                            channels=128, num_elems=512, num_idxs=NX8)
                        cv = cvp.tile([128, 256], f32, tag=f"cv{tag}", name=f"cv{tag}")
                        nc.scalar.activation(cv[:, :], fo[:, 0:256], AF.Copy)
                        pt = tp.tile([128, 256], f32, tag=f"pt{tag}", name=f"pt{tag}")
                        for half in range(2):
                            nc.tensor.transpose(
                                pt[:, half * 128:(half + 1) * 128],
                                cv[:, half * 128:(half + 1) * 128],
                                ident[:, :])
                        tr = trp.tile([128, 256], f32, tag=f"tr{tag}", name=f"tr{tag}")
                        nc.scalar.activation(tr[:, :], pt[:, :], AF.Copy)
                        oview = out_d.rearrange("(cc p) hh ww -> p cc hh ww", cc=2)
                        nc.sync.dma_start(
                            oview[:, :, h, :],
                            tr[:, :].rearrange("p (cc ww) -> p cc ww", ww=W))
    nc.compile()
    return nc


def _mask_params(mask):
    """Global (all-core) compile-time params from the actual mask."""
    mp = np.pad(mask, ((0, 0), (0, 0), (K, K), (K, K)))[:, 0]  # [B, H+16, W+16]
    n1 = np.zeros((B, H, W), np.int64)
    for dy in range(KW):
        for dx in range(KW):
            n1 += (mp[:, dy:dy + H, dx:dx + W] != 0)
    n0 = D - n1
    n1max, n0max = int(n1.max()), int(n0.max())
    WF = (n1max + 1) // 2 * 2
    WB = (n0max + 1) // 2 * 2
    # rounds: deepest kept slot is max(p, n-33) where p (#positives) <= n is
    # data-dependent; bound p by n/2 + 64 (many sigma for binomial signs).
    need_f = min(max(n1max - 33, n1max // 2 + 64), n1max)
    need_b = min(max(n0max - 33, n0max // 2 + 64), n0max)
    RF = (need_f + 7) // 8
    RB = (need_b + 7) // 8
    return WF, WB, RF, RB


def _host_prep(pre, curr, mask, params):
    WF, WB, RF, RB = params
    pre = np.asarray(pre, dtype=np.float32)
    curr = np.asarray(curr, dtype=np.float32)
    pre_pad = np.pad(pre, ((0, 0), (0, 0), (K, K), (K, K)), mode="reflect")
    mask_pad = np.pad(np.asarray(mask, np.float32), ((0, 0), (0, 0), (K, K), (K, K)))
    ins = []
    for k in range(N_CORES):
        b, hh = k // 2, k % 2
        h0 = hh * HSLICE
        mpk = mask_pad[b, 0, h0:h0 + HP, :]
        s0, s1 = mpk.strides
        m_unf = np.lib.stride_tricks.as_strided(
            mpk, (HSLICE, KW, W, KW), (s0, s0, s1, s1))
        m_unf = (m_unf.transpose(0, 2, 1, 3).reshape(HSLICE, W, D) != 0)
        # compaction scatter indices: dest rank among masked / unmasked
        csum1 = np.cumsum(m_unf, axis=-1, dtype=np.int32)
        rank1 = csum1 - m_unf  # exclusive cumsum
        d_ar = np.arange(D, dtype=np.int32)
        # rank among unmasked = d - (#masked in 0..d); valid at unmasked slots
        rank0 = d_ar[None, None, :] - csum1
        idx_f = np.where(m_unf, rank1, -1).astype(np.int16)
        idx_b = np.where(~m_unf, rank0, -1).astype(np.int16)
        pad_col = np.full((HSLICE, W, 1), -1, np.int16)
        idx_f = np.concatenate([idx_f, pad_col], axis=-1)
        idx_b = np.concatenate([idx_b, pad_col], axis=-1)
        ins.append({
            "pre_pad": np.ascontiguousarray(
                pre_pad[b, :, h0:h0 + HP, :]).astype(np.float16),
            "curr": np.ascontiguousarray(
                curr[b, :, h0:h0 + HSLICE, :]).astype(np.float16),
            "idx_f": np.ascontiguousarray(idx_f),
            "idx_b": np.ascontiguousarray(idx_b),
        })
    return ins


def kernel(pre, curr, mask, mode):
    from concourse.bass_utils import run_bass_kernel_spmd

    pre = np.asarray(pre, dtype=np.float32)
    curr = np.asarray(curr, dtype=np.float32)
    mask = np.asarray(mask, dtype=np.float32)
    assert int(np.asarray(mode)) == 0

    params = _mask_params(mask)
    if _CACHED.get("params") != params:
        _CACHED["nc"] = _build_nc(*params)
        _CACHED["params"] = params
    nc = _CACHED["nc"]

    in_maps = _host_prep(pre, curr, mask, params)
    res = run_bass_kernel_spmd(nc, in_maps, core_ids=list(range(N_CORES)))
    f = np.zeros((B, TOPK, H, W), np.float32)
    bo = np.zeros((B, TOPK, H, W), np.float32)
    for k in range(N_CORES):
        bb, hh = k // 2, k % 2
        f[bb, :, hh * HSLICE:(hh + 1) * HSLICE, :] = res.results[k]["f_out"]
        bo[bb, :, hh * HSLICE:(hh + 1) * HSLICE, :] = res.results[k]["b_out"]
    return (f, bo)


# revision 4
# speedup vs baseline: 2.5677x; 1.6152x over previous
"""Trainium2 Bass kernel for local-correlation + masked top-256 (sparse_attention).

Contract: kernel(**inputs) takes FULL unsharded inputs (pre, curr, mask, mode)
and returns the full output tuple (f, b), each [4, 256, 128, 128] f32.

Sharding: pure data parallel over (batch, H-half) -> 8 cores.

Algorithm per core (fp16 data path; rel-err budget 2e-2, achieved ~1e-3):
  - L2-normalize pre/curr over C (sumsq via ones-matmul in fp16 -> PSUM f32,
    inv = Exp(-0.5*Ln(ss)) with Ln/Exp in f32, scale applied in fp16).
  - Per output row h: 17 Gram matmuls (fp16, 1 cyc/col) -> PSUM; copied fp16
    into a staging tile; DRAM round-trip with write-row-stride 2449 /
    read-partition-stride 2450 extracts the diagonal band co[w, dy*17+dx].
  - Mask-compaction via gpsimd local_scatter with host-computed per-pixel
    rank indices: the nonzero entries of co*m / co*(1-m) land rank-ordered in
    a 256-slot array; unwritten slots are 0, which sort exactly where the
    true zeros belong.
  - Batched bitonic sort: 4 rows x 2 tensors = 8 independent 256-arrays in
    one [128, 2048] tile, descending, 36 compare-exchange stages of two
    tensor_tensor (max/min) DVE ops each, ping-ponged between two tiles.
  - Placement into the final [128, 256] output is a second local_scatter
    with dest = j + 33*[v<0] (33 = 289-256 accounts for the array's zero
    padding vs the true zero count; dst pre-zeroed by the instruction
    provides the zero block; overflow lands in slots [256, 290)).
  - fp16 -> f32 conversion, PE transpose [w,k] -> [k,w], DMA out.
"""

import numpy as np

K = 8
KW = 17
D = KW * KW            # 289
TOPK = 256
B, C, H, W = 4, 256, 128, 128
N_CORES = 8
HSLICE = H // 2        # 64 rows per core
WP = W + 2 * K         # 144
HP = HSLICE + 2 * K    # 80
PRE_COLS = HP * WP     # 11520
CUR_COLS = HSLICE * W  # 8192
STG = KW * WP          # 2448
SCR_FLAT = 128 * (STG + 2)   # divisible by 2450; first 128*2449 used for write view
NIC = D + 1            # 290: compaction scatter num_idxs (even)
RBATCH = 4             # rows per bitonic batch
NARR = RBATCH * 2      # sorted arrays per batch
SORTW = NARR * 256     # 2048

_CACHED = {}


def _bitonic_stages(n):
    """('flip'|'half', m_or_d) stages of a descending bitonic sort of n."""
    k = 1
    while (1 << k) <= n:
        m = 1 << k
        yield ('flip', m)
        d = m // 4
        while d >= 1:
            yield ('half', d)
            d //= 2
        k += 1


def _build_nc():
    import concourse.bacc as bacc
    import concourse.tile as tile
    import concourse.mybir as mybir

    f32 = mybir.dt.float32
    f16 = mybir.dt.float16
    i16 = mybir.dt.int16
    AF = mybir.ActivationFunctionType
    ALU = mybir.AluOpType

    nc = bacc.Bacc("TRN2", target_bir_lowering=False, debug=False,
                   enable_asserts=False, num_devices=N_CORES)

    pre_d = nc.dram_tensor("pre_pad", [C, HP, WP], f16, kind="ExternalInput").ap()
    cur_d = nc.dram_tensor("curr", [C, HSLICE, W], f16, kind="ExternalInput").ap()
    idxf_d = nc.dram_tensor("idx_f", [HSLICE, 128, NIC], i16, kind="ExternalInput").ap()
    idxb_d = nc.dram_tensor("idx_b", [HSLICE, 128, NIC], i16, kind="ExternalInput").ap()
    f_d = nc.dram_tensor("f_out", [TOPK, HSLICE, W], f32, kind="ExternalOutput").ap()
    b_d = nc.dram_tensor("b_out", [TOPK, HSLICE, W], f32, kind="ExternalOutput").ap()
    scr = [nc.dram_tensor(f"scr{i}", [SCR_FLAT], f16, kind="Internal").ap()
           for i in range(2)]

    ident_d = nc.inline_tensor(np.eye(128, dtype=np.float32), name="ident")
    ones_col_d = nc.inline_tensor(np.ones((128, 1), np.float16), name="ones_col")
    ones_row_d = nc.inline_tensor(np.ones((1, 128), np.float16), name="ones_row")
    iota_d = nc.inline_tensor(
        np.tile(np.arange(256, dtype=np.float16), (128, 1)), name="iota")

    # gram grouping: 17 dys in PSUM tiles of 3+3+3+3+3+2
    GRP = [(0, 3), (3, 3), (6, 3), (9, 3), (12, 3), (15, 2)]

    with tile.TileContext(nc) as tc:
        with tc.tile_pool(name="persist", bufs=1) as pp:
            pre_n = [pp.tile([128, PRE_COLS], f16, tag=f"pre{c}", name=f"pre{c}") for c in range(2)]
            cur_n = [pp.tile([128, CUR_COLS], f16, tag=f"cur{c}", name=f"cur{c}") for c in range(2)]
            ident = pp.tile([128, 128], f32, tag="ident", name="identt")
            ones_col = pp.tile([128, 1], f16, tag="onc", name="onc")
            ones_row = pp.tile([1, 128], f16, tag="onr", name="onr")
            iota = pp.tile([128, 256], f16, tag="iota", name="iota")
            nc.sync.dma_start(ident[:, :], ident_d.ap())
            nc.sync.dma_start(ones_col[:, :], ones_col_d.ap())
            nc.sync.dma_start(ones_row[:, :], ones_row_d.ap())
            nc.sync.dma_start(iota[:, :], iota_d.ap())
            for c in range(2):
                nc.sync.dma_start(
                    pre_n[c][:, :],
                    pre_d[c * 128:(c + 1) * 128, :, :].rearrange("c hh ww -> c (hh ww)"))
                nc.sync.dma_start(
                    cur_n[c][:, :],
                    cur_d[c * 128:(c + 1) * 128, :, :].rearrange("c hh ww -> c (hh ww)"))

            # ---- Stage A: L2 normalization over C (in place, fp16) ----
            with (tc.tile_pool(name="sqp", bufs=2) as sqp,
                  tc.tile_pool(name="strip", bufs=2) as stp,
                  tc.tile_pool(name="ssp", bufs=2, space="PSUM") as ssp,
                  tc.tile_pool(name="bcp", bufs=2, space="PSUM") as bcp):
                for tiles, ncols, cw in ((pre_n, PRE_COLS, 480), (cur_n, CUR_COLS, 512)):
                    for j in range(ncols // cw):
                        cs = slice(j * cw, (j + 1) * cw)
                        ss = ssp.tile([1, cw], f32, tag="ss", name="ss")
                        for c in range(2):
                            sq = sqp.tile([128, cw], f16, tag="sq", name="sq")
                            nc.scalar.activation(sq[:, :], tiles[c][:, cs], AF.Square)
                            nc.tensor.matmul(ss[:, :], ones_col[:, :], sq[:, :],
                                             start=(c == 0), stop=(c == 1))
                        lns = stp.tile([1, cw], f32, tag="lns", name="lns")
                        nc.scalar.activation(lns[:, :], ss[:, :], AF.Ln)
                        inv1 = stp.tile([1, cw], f16, tag="inv1", name="inv1")
                        nc.scalar.activation(inv1[:, :], lns[:, :], AF.Exp, scale=-0.5)
                        bc = bcp.tile([128, cw], f32, tag="bc", name="bc")
                        nc.tensor.matmul(bc[:, :], ones_row[:, :], inv1[:, :],
                                         start=True, stop=True)
                        inv = sqp.tile([128, cw], f16, tag="inv", name="inv")
                        nc.scalar.activation(inv[:, :], bc[:, :], AF.Copy)
                        for c in range(2):
                            nc.gpsimd.tensor_tensor(
                                out=tiles[c][:, cs], in0=tiles[c][:, cs],
                                in1=inv[:, :], op=ALU.mult)

            # ---- Stage B: per-row gram, shear, compact; batched sort; place ----
            with (tc.tile_pool(name="stage", bufs=2) as stgp,
                  tc.tile_pool(name="cop", bufs=2) as cop,
                  tc.tile_pool(name="idxp", bufs=2) as idxp,
                  tc.tile_pool(name="xsp", bufs=2) as xsp,
                  tc.tile_pool(name="ysp", bufs=2) as ysp,
                  tc.tile_pool(name="dstp", bufs=2) as dstp,
                  tc.tile_pool(name="fop", bufs=2) as fop,
                  tc.tile_pool(name="cvp", bufs=2) as cvp,
                  tc.tile_pool(name="trp", bufs=2) as trp,
                  tc.tile_pool(name="gp", bufs=1, space="PSUM") as gp,
                  tc.tile_pool(name="tp", bufs=1, space="PSUM") as tp):
                for hb in range(0, HSLICE, RBATCH):
                    xs = xsp.tile([128, SORTW], f16, tag="xs", name="xs")
                    ys = ysp.tile([128, SORTW], f16, tag="ys", name="ys")
                    for hr in range(RBATCH):
                        h = hb + hr
                        stage = stgp.tile([128, STG], f16, tag="stage", name="stage")
                        for gi, (dy0, ndy) in enumerate(GRP):
                            g = gp.tile([128, ndy * WP], f32, tag=f"g{gi}", name=f"g{gi}")
                            for li in range(ndy):
                                dy = dy0 + li
                                for c in range(2):
                                    nc.tensor.matmul(
                                        g[:, li * WP:(li + 1) * WP],
                                        cur_n[c][:, h * W:(h + 1) * W],
                                        pre_n[c][:, (h + dy) * WP:(h + dy + 1) * WP],
                                        start=(c == 0), stop=(c == 1))
                            nc.scalar.activation(
                                stage[:, dy0 * WP:(dy0 + ndy) * WP], g[:, :], AF.Copy)
                        sc = scr[h % 2]
                        wview = sc[0:128 * (STG + 1)].rearrange("(p r) -> p r", r=STG + 1)
                        nc.sync.dma_start(wview[:, 0:STG], stage[:, :])
                        co = cop.tile([128, NIC], f16, tag="co", name="co")
                        rview = sc[:].rearrange("(p r) -> p r", r=STG + 2)
                        rview = rview[:, 0:STG].rearrange("p (a b) -> p a b", b=WP)
                        nc.sync.dma_start(co[:, 0:D], rview[:, :, 0:KW])
                        nc.gpsimd.memset(co[:, D:NIC], -3.0)

                        idxf = idxp.tile([128, NIC], i16, tag="idxf", name="idxf")
                        idxb = idxp.tile([128, NIC], i16, tag="idxb", name="idxb")
                        nc.sync.dma_start(idxf[:, :], idxf_d[h, :, :])
                        nc.sync.dma_start(idxb[:, :], idxb_d[h, :, :])
                        for t, idxt in ((0, idxf), (1, idxb)):
                            arr = hr * 2 + t
                            nc.gpsimd.local_scatter(
                                xs[:, arr * 256:(arr + 1) * 256], co[:, :], idxt[:, :],
                                channels=128, num_elems=256, num_idxs=NIC)

                    # ---- batched descending bitonic sort of 8 x 256 ----
                    cur_t, nxt_t = xs, ys
                    for kind, v in _bitonic_stages(256):
                        if kind == 'flip':
                            hh = v // 2
                            cv_ = cur_t[:, :].rearrange(
                                "p (A two hh) -> p A two hh", two=2, hh=hh)
                            nv_ = nxt_t[:, :].rearrange(
                                "p (A two hh) -> p A two hh", two=2, hh=hh)
                            lo_i, hi_i = cv_[:, :, 0, :], cv_[:, :, 1, ::-1]
                            lo_o, hi_o = nv_[:, :, 0, :], nv_[:, :, 1, ::-1]
                        else:
                            dd = v
                            cv_ = cur_t[:, :].rearrange(
                                "p (A two dd) -> p A two dd", two=2, dd=dd)
                            nv_ = nxt_t[:, :].rearrange(
                                "p (A two dd) -> p A two dd", two=2, dd=dd)
                            lo_i, hi_i = cv_[:, :, 0, :], cv_[:, :, 1, :]
                            lo_o, hi_o = nv_[:, :, 0, :], nv_[:, :, 1, :]
                        nc.vector.tensor_tensor(out=lo_o, in0=lo_i, in1=hi_i, op=ALU.max)
                        nc.vector.tensor_tensor(out=hi_o, in0=lo_i, in1=hi_i, op=ALU.min)
                        cur_t, nxt_t = nxt_t, cur_t
                    # 36 stages (even) -> result back in xs

                    for hr in range(RBATCH):
                        h = hb + hr
                        for t, out_d, tag in ((0, f_d, "f"), (1, b_d, "b")):
                            arr = hr * 2 + t
                            st = xs[:, arr * 256:(arr + 1) * 256]
                            # dest = j + 33*[v<0]; zero-pad vs true-zero delta
                            ng = dstp.tile([128, 256], f16, tag=f"ng{tag}", name=f"ng{tag}")
                            nc.gpsimd.tensor_scalar(
                                out=ng[:, :], in0=st,
                                scalar1=0.0, scalar2=float(D - 256),
                                op0=ALU.is_lt, op1=ALU.mult)
                            nc.gpsimd.tensor_tensor(
                                out=ng[:, :], in0=ng[:, :], in1=iota[:, :], op=ALU.add)
                            di = dstp.tile([128, 256], i16, tag=f"di{tag}", name=f"di{tag}")
                            nc.scalar.activation(di[:, :], ng[:, :], AF.Copy)
                            fo = fop.tile([128, 290], f16, tag=f"fo{tag}", name=f"fo{tag}")
                            nc.gpsimd.local_scatter(
                                fo[:, :], st, di[:, :],
                                channels=128, num_elems=290, num_idxs=256)
                            cv = cvp.tile([128, 256], f32, tag=f"cv{tag}", name=f"cv{tag}")
                            nc.scalar.activation(cv[:, :], fo[:, 0:256], AF.Copy)
                            pt = tp.tile([128, 256], f32, tag=f"pt{tag}", name=f"pt{tag}")
                            for half in range(2):
                                nc.tensor.transpose(
                                    pt[:, half * 128:(half + 1) * 128],
                                    cv[:, half * 128:(half + 1) * 128],
                                    ident[:, :])
                            tr = trp.tile([128, 256], f32, tag=f"tr{tag}", name=f"tr{tag}")
                            nc.scalar.activation(tr[:, :], pt[:, :], AF.Copy)
                            oview = out_d.rearrange("(cc p) hh ww -> p cc hh ww", cc=2)
                            nc.sync.dma_start(
                                oview[:, :, h, :],
                                tr[:, :].rearrange("p (cc ww) -> p cc ww", ww=W))
    nc.compile()
    return nc


def _mask_params(mask):
    """Sanity-check the mask against the fixed 256-slot sort arrays."""
    mp = np.pad(mask, ((0, 0), (0, 0), (K, K), (K, K)))[:, 0]
    n1 = np.zeros((B, H, W), np.int64)
    for dy in range(KW):
        for dx in range(KW):
            n1 += (mp[:, dy:dy + H, dx:dx + W] != 0)
    assert int(n1.max()) <= 256 and int((D - n1).max()) <= 256, (
        "mask window occupancy exceeds 256; fixed-width sort arrays too small")
    return ("v2",)


def _host_prep(pre, curr, mask, params=None):
    pre = np.asarray(pre, dtype=np.float32)
    curr = np.asarray(curr, dtype=np.float32)
    pre_pad = np.pad(pre, ((0, 0), (0, 0), (K, K), (K, K)), mode="reflect")
    mask_pad = np.pad(np.asarray(mask, np.float32), ((0, 0), (0, 0), (K, K), (K, K)))
    ins = []
    for k in range(N_CORES):
        b, hh = k // 2, k % 2
        h0 = hh * HSLICE
        mpk = mask_pad[b, 0, h0:h0 + HP, :]
        s0, s1 = mpk.strides
        m_unf = np.lib.stride_tricks.as_strided(
            mpk, (HSLICE, KW, W, KW), (s0, s0, s1, s1))
        m_unf = (m_unf.transpose(0, 2, 1, 3).reshape(HSLICE, W, D) != 0)
        # compaction scatter indices: dest rank among masked / unmasked
        csum1 = np.cumsum(m_unf, axis=-1, dtype=np.int32)
        rank1 = csum1 - m_unf  # exclusive cumsum
        d_ar = np.arange(D, dtype=np.int32)
        # rank among unmasked = d - (#masked in 0..d); valid at unmasked slots
        rank0 = d_ar[None, None, :] - csum1
        idx_f = np.where(m_unf, rank1, -1).astype(np.int16)
        idx_b = np.where(~m_unf, rank0, -1).astype(np.int16)
        pad_col = np.full((HSLICE, W, 1), -1, np.int16)
        idx_f = np.concatenate([idx_f, pad_col], axis=-1)
        idx_b = np.concatenate([idx_b, pad_col], axis=-1)
        ins.append({
            "pre_pad": np.ascontiguousarray(
                pre_pad[b, :, h0:h0 + HP, :]).astype(np.float16),
            "curr": np.ascontiguousarray(
                curr[b, :, h0:h0 + HSLICE, :]).astype(np.float16),
            "idx_f": np.ascontiguousarray(idx_f),
            "idx_b": np.ascontiguousarray(idx_b),
        })
    return ins


def kernel(pre, curr, mask, mode):
    from concourse.bass_utils import run_bass_kernel_spmd

    pre = np.asarray(pre, dtype=np.float32)
    curr = np.asarray(curr, dtype=np.float32)
    mask = np.asarray(mask, dtype=np.float32)
    assert int(np.asarray(mode)) == 0

    params = _mask_params(mask)
    if _CACHED.get("params") != params:
        _CACHED["nc"] = _build_nc()
        _CACHED["params"] = params
    nc = _CACHED["nc"]

    in_maps = _host_prep(pre, curr, mask, params)
    res = run_bass_kernel_spmd(nc, in_maps, core_ids=list(range(N_CORES)))
    f = np.zeros((B, TOPK, H, W), np.float32)
    bo = np.zeros((B, TOPK, H, W), np.float32)
    for k in range(N_CORES):
        bb, hh = k // 2, k % 2
        f[bb, :, hh * HSLICE:(hh + 1) * HSLICE, :] = res.results[k]["f_out"]
        bo[bb, :, hh * HSLICE:(hh + 1) * HSLICE, :] = res.results[k]["b_out"]
    return (f, bo)


# revision 12
# speedup vs baseline: 2.9589x; 1.1523x over previous
"""Trainium2 Bass kernel for local-correlation + masked top-256 (sparse_attention).

Contract: kernel(**inputs) takes FULL unsharded inputs (pre, curr, mask, mode)
and returns the full output tuple (f, b), each [4, 256, 128, 128] f32.

Sharding: pure data parallel over (batch, H-half) -> 8 cores.

Algorithm per core (fp16 data path; rel-err budget 2e-2, achieved ~1e-3):
  - L2-normalize pre/curr over C (sumsq via ones-matmul in fp16 -> PSUM f32,
    inv = Exp(-0.5*Ln(ss)) with Ln/Exp in f32, scale applied in fp16).
  - Per output row h: 17 Gram matmuls (fp16, 1 cyc/col) -> PSUM; copied fp16
    into a staging tile; DRAM round-trip with write-row-stride 2449 /
    read-partition-stride 2450 extracts the diagonal band co[w, dy*17+dx].
  - Mask-compaction via gpsimd local_scatter with host-computed per-pixel
    rank indices: the nonzero entries of co*m / co*(1-m) land rank-ordered in
    a 256-slot array; unwritten slots are 0, which sort exactly where the
    true zeros belong.
  - Batched bitonic sort: 4 rows x 2 tensors = 8 independent 256-arrays in
    one [128, 2048] tile, descending, 36 compare-exchange stages of two
    tensor_tensor (max/min) DVE ops each, ping-ponged between two tiles.
  - Placement into the final [128, 256] output is a second local_scatter
    with dest = j + 33*[v<0] (33 = 289-256 accounts for the array's zero
    padding vs the true zero count; dst pre-zeroed by the instruction
    provides the zero block; overflow lands in slots [256, 290)).
  - fp16 -> f32 conversion, PE transpose [w,k] -> [k,w], DMA out.
"""

import numpy as np

K = 8
KW = 17
D = KW * KW            # 289
TOPK = 256
B, C, H, W = 4, 256, 128, 128
N_CORES = 8
HSLICE = H // 2        # 64 rows per core
WP = W + 2 * K         # 144
HP = HSLICE + 2 * K    # 80
PRE_COLS = HP * WP     # 11520
CUR_COLS = HSLICE * W  # 8192
STG = KW * WP          # 2448
SCR_FLAT = 128 * (STG + 2)   # divisible by 2450; first 128*2449 used for write view
NIC = D + 1            # 290: compaction scatter num_idxs (even)
RBATCH = 4             # rows per bitonic batch
NARR = RBATCH * 2      # sorted arrays per batch
SORTW = NARR * 256     # 2048

_CACHED = {}


def _bitonic_stages(n):
    """('flip'|'half', m_or_d) stages of a descending bitonic sort of n."""
    k = 1
    while (1 << k) <= n:
        m = 1 << k
        yield ('flip', m)
        d = m // 4
        while d >= 1:
            yield ('half', d)
            d //= 2
        k += 1


def _build_nc():
    import concourse.bacc as bacc
    import concourse.tile as tile
    import concourse.mybir as mybir

    f32 = mybir.dt.float32
    f16 = mybir.dt.float16
    i16 = mybir.dt.int16
    AF = mybir.ActivationFunctionType
    ALU = mybir.AluOpType

    nc = bacc.Bacc("TRN2", target_bir_lowering=False, debug=False,
                   enable_asserts=False, num_devices=N_CORES)

    pre_d = nc.dram_tensor("pre_pad", [C, HP, WP], f16, kind="ExternalInput").ap()
    cur_d = nc.dram_tensor("curr", [C, HSLICE, W], f16, kind="ExternalInput").ap()
    idxf_d = nc.dram_tensor("idx_f", [HSLICE, 128, NIC], i16, kind="ExternalInput").ap()
    idxb_d = nc.dram_tensor("idx_b", [HSLICE, 128, NIC], i16, kind="ExternalInput").ap()
    f_d = nc.dram_tensor("f_out", [TOPK, HSLICE, W], f32, kind="ExternalOutput").ap()
    b_d = nc.dram_tensor("b_out", [TOPK, HSLICE, W], f32, kind="ExternalOutput").ap()
    scr = [nc.dram_tensor(f"scr{i}", [SCR_FLAT], f16, kind="Internal").ap()
           for i in range(4)]

    ident_d = nc.inline_tensor(np.eye(128, dtype=np.float32), name="ident")
    ones_col_d = nc.inline_tensor(np.ones((128, 1), np.float16), name="ones_col")
    ones_row_d = nc.inline_tensor(np.ones((1, 128), np.float16), name="ones_row")
    iota_d = nc.inline_tensor(
        np.tile(np.arange(256, dtype=np.float16), (128, 1)), name="iota")

    # gram grouping: 17 dys in PSUM tiles of 3+3+3+3+3+2
    GRP = [(0, 3), (3, 3), (6, 3), (9, 3), (12, 3), (15, 2)]

    with tile.TileContext(nc) as tc:
        with tc.tile_pool(name="persist", bufs=1) as pp:
            pre_n = [pp.tile([128, PRE_COLS], f16, tag=f"pre{c}", name=f"pre{c}") for c in range(2)]
            cur_n = [pp.tile([128, CUR_COLS], f16, tag=f"cur{c}", name=f"cur{c}") for c in range(2)]
            ident = pp.tile([128, 128], f32, tag="ident", name="identt")
            ones_col = pp.tile([128, 1], f16, tag="onc", name="onc")
            ones_row = pp.tile([1, 128], f16, tag="onr", name="onr")
            iota = pp.tile([128, 256], f16, tag="iota", name="iota")
            nc.sync.dma_start(ident[:, :], ident_d.ap())
            nc.sync.dma_start(ones_col[:, :], ones_col_d.ap())
            nc.sync.dma_start(ones_row[:, :], ones_row_d.ap())
            nc.sync.dma_start(iota[:, :], iota_d.ap())
            pre_flat = [pre_d[c * 128:(c + 1) * 128, :, :].rearrange(
                "c hh ww -> c (hh ww)") for c in range(2)]
            cur_flat = [cur_d[c * 128:(c + 1) * 128, :, :].rearrange(
                "c hh ww -> c (hh ww)") for c in range(2)]

            # ---- Stage A: L2 normalization over C (in place, fp16) ----
            # per-chunk input DMA + Square/ones-matmul sumsq + one-op
            # Abs_reciprocal_sqrt (all funcs in one act table set)
            with (tc.tile_pool(name="sqp", bufs=2) as sqp,
                  tc.tile_pool(name="strip", bufs=2) as stp,
                  tc.tile_pool(name="ssp", bufs=2, space="PSUM") as ssp,
                  tc.tile_pool(name="bcp", bufs=2, space="PSUM") as bcp):
                for tiles, flats, ncols, cw in (
                        (pre_n, pre_flat, PRE_COLS, 480),
                        (cur_n, cur_flat, CUR_COLS, 512)):
                    for j in range(ncols // cw):
                        cs = slice(j * cw, (j + 1) * cw)
                        ss = ssp.tile([1, cw], f32, tag="ss", name="ss")
                        for c in range(2):
                            nc.sync.dma_start(tiles[c][:, cs], flats[c][:, cs])
                            sq = sqp.tile([128, cw], f16, tag="sq", name="sq")
                            nc.scalar.activation(sq[:, :], tiles[c][:, cs], AF.Square)
                            nc.tensor.matmul(ss[:, :], ones_col[:, :], sq[:, :],
                                             start=(c == 0), stop=(c == 1))
                        inv1 = stp.tile([1, cw], f16, tag="inv1", name="inv1")
                        nc.scalar.activation(inv1[:, :], ss[:, :],
                                             AF.Abs_reciprocal_sqrt)
                        bc = bcp.tile([128, cw], f32, tag="bc", name="bc")
                        nc.tensor.matmul(bc[:, :], ones_row[:, :], inv1[:, :],
                                         start=True, stop=True)
                        inv = sqp.tile([128, cw], f16, tag="inv", name="inv")
                        nc.scalar.activation(inv[:, :], bc[:, :], AF.Copy)
                        for c in range(2):
                            nc.gpsimd.tensor_tensor(
                                out=tiles[c][:, cs], in0=tiles[c][:, cs],
                                in1=inv[:, :], op=ALU.mult)

            # ---- Stage B: per-row gram, shear, compact; batched sort; place ----
            with (tc.tile_pool(name="stage", bufs=3) as stgp,
                  tc.tile_pool(name="cop", bufs=3) as cop,
                  tc.tile_pool(name="idxp", bufs=3) as idxp,
                  tc.tile_pool(name="xsp", bufs=2) as xsp,
                  tc.tile_pool(name="ysp", bufs=2) as ysp,
                  tc.tile_pool(name="dstp", bufs=2) as dstp,
                  tc.tile_pool(name="fop", bufs=2) as fop,
                  tc.tile_pool(name="cvp", bufs=2) as cvp,
                  tc.tile_pool(name="trp", bufs=2) as trp,
                  tc.tile_pool(name="gp", bufs=1, space="PSUM") as gp,
                  tc.tile_pool(name="tp", bufs=1, space="PSUM") as tp):
                for hb in range(0, HSLICE, RBATCH):
                    xs = xsp.tile([128, SORTW], f16, tag="xs", name="xs")
                    ys = ysp.tile([128, SORTW], f16, tag="ys", name="ys")
                    for hr in range(RBATCH):
                        h = hb + hr
                        sc = scr[h % 4]
                        stage = stgp.tile([128, STG], f16, tag="stage", name="stage")
                        for gi, (dy0, ndy) in enumerate(GRP):
                            g = gp.tile([128, ndy * WP], f32, tag=f"g{gi}", name=f"g{gi}")
                            for li in range(ndy):
                                dy = dy0 + li
                                for c in range(2):
                                    nc.tensor.matmul(
                                        g[:, li * WP:(li + 1) * WP],
                                        cur_n[c][:, h * W:(h + 1) * W],
                                        pre_n[c][:, (h + dy) * WP:(h + dy + 1) * WP],
                                        start=(c == 0), stop=(c == 1))
                            nc.scalar.activation(
                                stage[:, dy0 * WP:(dy0 + ndy) * WP], g[:, :], AF.Copy)
                        wview = sc[0:128 * (STG + 1)].rearrange("(p r) -> p r", r=STG + 1)
                        nc.sync.dma_start(wview[:, 0:STG], stage[:, :])
                        co = cop.tile([128, NIC], f16, tag="co", name="co")
                        rview = sc[:].rearrange("(p r) -> p r", r=STG + 2)
                        rview = rview[:, 0:STG].rearrange("p (a b) -> p a b", b=WP)
                        nc.sync.dma_start(co[:, 0:D], rview[:, :, 0:KW])
                        nc.gpsimd.memset(co[:, D:NIC], -3.0)

                        idxf = idxp.tile([128, NIC], i16, tag="idxf", name="idxf")
                        idxb = idxp.tile([128, NIC], i16, tag="idxb", name="idxb")
                        nc.sync.dma_start(idxf[:, :], idxf_d[h, :, :])
                        nc.sync.dma_start(idxb[:, :], idxb_d[h, :, :])
                        for t, idxt in ((0, idxf), (1, idxb)):
                            arr = hr * 2 + t
                            nc.gpsimd.local_scatter(
                                xs[:, arr * 256:(arr + 1) * 256], co[:, :], idxt[:, :],
                                channels=128, num_elems=256, num_idxs=NIC)

                    # ---- batched descending bitonic sort of 8 x 256 ----
                    cur_t, nxt_t = xs, ys
                    for kind, v in _bitonic_stages(256):
                        if kind == 'flip':
                            hh = v // 2
                            cv_ = cur_t[:, :].rearrange(
                                "p (A two hh) -> p A two hh", two=2, hh=hh)
                            nv_ = nxt_t[:, :].rearrange(
                                "p (A two hh) -> p A two hh", two=2, hh=hh)
                            lo_i, hi_i = cv_[:, :, 0, :], cv_[:, :, 1, ::-1]
                            lo_o, hi_o = nv_[:, :, 0, :], nv_[:, :, 1, ::-1]
                        else:
                            dd = v
                            cv_ = cur_t[:, :].rearrange(
                                "p (A two dd) -> p A two dd", two=2, dd=dd)
                            nv_ = nxt_t[:, :].rearrange(
                                "p (A two dd) -> p A two dd", two=2, dd=dd)
                            lo_i, hi_i = cv_[:, :, 0, :], cv_[:, :, 1, :]
                            lo_o, hi_o = nv_[:, :, 0, :], nv_[:, :, 1, :]
                        nc.vector.tensor_tensor(out=lo_o, in0=lo_i, in1=hi_i, op=ALU.max)
                        nc.vector.tensor_tensor(out=hi_o, in0=lo_i, in1=hi_i, op=ALU.min)
                        cur_t, nxt_t = nxt_t, cur_t
                    # 36 stages (even) -> result back in xs

                    for hr in range(RBATCH):
                        h = hb + hr
                        for t, out_d, tag in ((0, f_d, "f"), (1, b_d, "b")):
                            arr = hr * 2 + t
                            st = xs[:, arr * 256:(arr + 1) * 256]
                            # dest = j + 33*[v<0]; zero-pad vs true-zero delta
                            ng = dstp.tile([128, 256], f16, tag=f"ng{tag}", name=f"ng{tag}")
                            nc.gpsimd.tensor_scalar(
                                out=ng[:, :], in0=st,
                                scalar1=0.0, scalar2=float(D - 256),
                                op0=ALU.is_lt, op1=ALU.mult)
                            nc.gpsimd.tensor_tensor(
                                out=ng[:, :], in0=ng[:, :], in1=iota[:, :], op=ALU.add)
                            di = dstp.tile([128, 256], i16, tag=f"di{tag}", name=f"di{tag}")
                            nc.scalar.activation(di[:, :], ng[:, :], AF.Copy)
                            fo = fop.tile([128, 290], f16, tag=f"fo{tag}", name=f"fo{tag}")
                            nc.gpsimd.local_scatter(
                                fo[:, :], st, di[:, :],
                                channels=128, num_elems=290, num_idxs=256)
                            cv = cvp.tile([128, 256], f32, tag=f"cv{tag}", name=f"cv{tag}")
                            nc.scalar.activation(cv[:, :], fo[:, 0:256], AF.Copy)
                            pt = tp.tile([128, 256], f32, tag=f"pt{tag}", name=f"pt{tag}")
                            for half in range(2):
                                nc.tensor.transpose(
                                    pt[:, half * 128:(half + 1) * 128],
                                    cv[:, half * 128:(half + 1) * 128],
                                    ident[:, :])
                            tr = trp.tile([128, 256], f32, tag=f"tr{tag}", name=f"tr{tag}")
                            nc.scalar.activation(tr[:, :], pt[:, :], AF.Copy)
                            oview = out_d.rearrange("(cc p) hh ww -> p cc hh ww", cc=2)
                            nc.sync.dma_start(
                                oview[:, :, h, :],
                                tr[:, :].rearrange("p (cc ww) -> p cc ww", ww=W))
    nc.compile()
    return nc


def _mask_params(mask):
    """Sanity-check the mask against the fixed 256-slot sort arrays."""
    mp = np.pad(mask, ((0, 0), (0, 0), (K, K), (K, K)))[:, 0]
    n1 = np.zeros((B, H, W), np.int64)
    for dy in range(KW):
        for dx in range(KW):
            n1 += (mp[:, dy:dy + H, dx:dx + W] != 0)
    assert int(n1.max()) <= 256 and int((D - n1).max()) <= 256, (
        "mask window occupancy exceeds 256; fixed-width sort arrays too small")
    return ("v2",)


def _host_prep(pre, curr, mask, params=None):
    pre = np.asarray(pre, dtype=np.float32)
    curr = np.asarray(curr, dtype=np.float32)
    pre_pad = np.pad(pre, ((0, 0), (0, 0), (K, K), (K, K)), mode="reflect")
    mask_pad = np.pad(np.asarray(mask, np.float32), ((0, 0), (0, 0), (K, K), (K, K)))
    ins = []
    for k in range(N_CORES):
        b, hh = k // 2, k % 2
        h0 = hh * HSLICE
        mpk = mask_pad[b, 0, h0:h0 + HP, :]
        s0, s1 = mpk.strides
        m_unf = np.lib.stride_tricks.as_strided(
            mpk, (HSLICE, KW, W, KW), (s0, s0, s1, s1))
        m_unf = (m_unf.transpose(0, 2, 1, 3).reshape(HSLICE, W, D) != 0)
        # compaction scatter indices: dest rank among masked / unmasked
        csum1 = np.cumsum(m_unf, axis=-1, dtype=np.int32)
        rank1 = csum1 - m_unf  # exclusive cumsum
        d_ar = np.arange(D, dtype=np.int32)
        # rank among unmasked = d - (#masked in 0..d); valid at unmasked slots
        rank0 = d_ar[None, None, :] - csum1
        idx_f = np.where(m_unf, rank1, -1).astype(np.int16)
        idx_b = np.where(~m_unf, rank0, -1).astype(np.int16)
        pad_col = np.full((HSLICE, W, 1), -1, np.int16)
        idx_f = np.concatenate([idx_f, pad_col], axis=-1)
        idx_b = np.concatenate([idx_b, pad_col], axis=-1)
        ins.append({
            "pre_pad": np.ascontiguousarray(
                pre_pad[b, :, h0:h0 + HP, :]).astype(np.float16),
            "curr": np.ascontiguousarray(
                curr[b, :, h0:h0 + HSLICE, :]).astype(np.float16),
            "idx_f": np.ascontiguousarray(idx_f),
            "idx_b": np.ascontiguousarray(idx_b),
        })
    return ins


def kernel(pre, curr, mask, mode):
    from concourse.bass_utils import run_bass_kernel_spmd

    pre = np.asarray(pre, dtype=np.float32)
    curr = np.asarray(curr, dtype=np.float32)
    mask = np.asarray(mask, dtype=np.float32)
    assert int(np.asarray(mode)) == 0

    params = _mask_params(mask)
    if _CACHED.get("params") != params:
        _CACHED["nc"] = _build_nc()
        _CACHED["params"] = params
    nc = _CACHED["nc"]

    in_maps = _host_prep(pre, curr, mask, params)
    res = run_bass_kernel_spmd(nc, in_maps, core_ids=list(range(N_CORES)))
    f = np.zeros((B, TOPK, H, W), np.float32)
    bo = np.zeros((B, TOPK, H, W), np.float32)
    for k in range(N_CORES):
        bb, hh = k // 2, k % 2
        f[bb, :, hh * HSLICE:(hh + 1) * HSLICE, :] = res.results[k]["f_out"]
        bo[bb, :, hh * HSLICE:(hh + 1) * HSLICE, :] = res.results[k]["b_out"]
    return (f, bo)


# revision 17
# speedup vs baseline: 3.2749x; 1.1068x over previous
"""Trainium2 Bass kernel for local-correlation + masked top-256 (sparse_attention).

Contract: kernel(**inputs) takes FULL unsharded inputs (pre, curr, mask, mode)
and returns the full output tuple (f, b), each [4, 256, 128, 128] f32.

Sharding: pure data parallel over (batch, H-half) -> 8 cores.

Algorithm per core (fp16 data path; rel-err budget 2e-2, achieved ~1e-3):
  - L2-normalize pre/curr over C (sumsq via ones-matmul in fp16 -> PSUM f32,
    inv = Exp(-0.5*Ln(ss)) with Ln/Exp in f32, scale applied in fp16).
  - Per output row h: 17 Gram matmuls (fp16, 1 cyc/col) -> PSUM; copied fp16
    into a staging tile; DRAM round-trip with write-row-stride 2449 /
    read-partition-stride 2450 extracts the diagonal band co[w, dy*17+dx].
  - Mask-compaction via gpsimd local_scatter with host-computed per-pixel
    rank indices: the nonzero entries of co*m / co*(1-m) land rank-ordered in
    a 256-slot array; unwritten slots are 0, which sort exactly where the
    true zeros belong.
  - Batched bitonic sort: 4 rows x 2 tensors = 8 independent 256-arrays in
    one [128, 2048] tile, descending, 36 compare-exchange stages of two
    tensor_tensor (max/min) DVE ops each, ping-ponged between two tiles.
  - Placement into the final [128, 256] output is a second local_scatter
    with dest = j + 33*[v<0] (33 = 289-256 accounts for the array's zero
    padding vs the true zero count; dst pre-zeroed by the instruction
    provides the zero block; overflow lands in slots [256, 290)).
  - fp16 -> f32 conversion, PE transpose [w,k] -> [k,w], DMA out.
"""

import numpy as np

K = 8
KW = 17
D = KW * KW            # 289
TOPK = 256
B, C, H, W = 4, 256, 128, 128
N_CORES = 8
HSLICE = H // 2        # 64 rows per core
WP = W + 2 * K         # 144
HP = HSLICE + 2 * K    # 80
PRE_COLS = HP * WP     # 11520
CUR_COLS = HSLICE * W  # 8192
STG = KW * WP          # 2448
SCR_FLAT = 128 * (STG + 2)   # divisible by 2450; first 128*2449 used for write view
NIC = D + 1            # 290: compaction scatter num_idxs (even)
RBATCH = 4             # rows per bitonic batch
NARR = RBATCH * 2      # sorted arrays per batch
SORTW = NARR * 256     # 2048

_CACHED = {}


def _bitonic_stages(n):
    """('flip'|'half', m_or_d) stages of a descending bitonic sort of n."""
    k = 1
    while (1 << k) <= n:
        m = 1 << k
        yield ('flip', m)
        d = m // 4
        while d >= 1:
            yield ('half', d)
            d //= 2
        k += 1


def _build_nc():
    import concourse.bacc as bacc
    import concourse.tile as tile
    import concourse.mybir as mybir

    f32 = mybir.dt.float32
    f16 = mybir.dt.float16
    i16 = mybir.dt.int16
    AF = mybir.ActivationFunctionType
    ALU = mybir.AluOpType

    nc = bacc.Bacc("TRN2", target_bir_lowering=False, debug=False,
                   enable_asserts=False, num_devices=N_CORES)

    pre_d = nc.dram_tensor("pre_pad", [C, HP, WP], f16, kind="ExternalInput").ap()
    cur_d = nc.dram_tensor("curr", [C, HSLICE, W], f16, kind="ExternalInput").ap()
    idxf_d = nc.dram_tensor("idx_f", [HSLICE, 128, NIC], i16, kind="ExternalInput").ap()
    idxb_d = nc.dram_tensor("idx_b", [HSLICE, 128, NIC], i16, kind="ExternalInput").ap()
    f_d = nc.dram_tensor("f_out", [TOPK, HSLICE, W], f32, kind="ExternalOutput").ap()
    b_d = nc.dram_tensor("b_out", [TOPK, HSLICE, W], f32, kind="ExternalOutput").ap()
    scr = [nc.dram_tensor(f"scr{i}", [SCR_FLAT], f16, kind="Internal").ap()
           for i in range(4)]

    ident_d = nc.inline_tensor(np.eye(128, dtype=np.float32), name="ident")
    ones_col_d = nc.inline_tensor(np.ones((128, 1), np.float16), name="ones_col")
    ones_row_d = nc.inline_tensor(np.ones((1, 128), np.float16), name="ones_row")
    iota_d = nc.inline_tensor(
        np.tile(np.arange(256, dtype=np.float16), (128, 1)), name="iota")

    # gram grouping: 17 dys in PSUM tiles of 3+3+3+3+3+2
    GRP = [(0, 3), (3, 3), (6, 3), (9, 3), (12, 3), (15, 2)]

    with tile.TileContext(nc) as tc:
        with tc.tile_pool(name="persist", bufs=1) as pp:
            pre_n = [pp.tile([128, PRE_COLS], f16, tag=f"pre{c}", name=f"pre{c}") for c in range(2)]
            cur_n = [pp.tile([128, CUR_COLS], f16, tag=f"cur{c}", name=f"cur{c}") for c in range(2)]
            ident = pp.tile([128, 128], f32, tag="ident", name="identt")
            ones_col = pp.tile([128, 1], f16, tag="onc", name="onc")
            ones_row = pp.tile([1, 128], f16, tag="onr", name="onr")
            iota = pp.tile([128, 256], f16, tag="iota", name="iota")
            nc.sync.dma_start(ident[:, :], ident_d.ap())
            nc.sync.dma_start(ones_col[:, :], ones_col_d.ap())
            nc.sync.dma_start(ones_row[:, :], ones_row_d.ap())
            nc.sync.dma_start(iota[:, :], iota_d.ap())
            pre_flat = [pre_d[c * 128:(c + 1) * 128, :, :].rearrange(
                "c hh ww -> c (hh ww)") for c in range(2)]
            cur_flat = [cur_d[c * 128:(c + 1) * 128, :, :].rearrange(
                "c hh ww -> c (hh ww)") for c in range(2)]

            # ---- Stage A: L2 normalization over C (in place, fp16) ----
            # per-chunk input DMA + Square/ones-matmul sumsq + one-op
            # Abs_reciprocal_sqrt (all funcs in one act table set)
            with (tc.tile_pool(name="sqp", bufs=2) as sqp,
                  tc.tile_pool(name="strip", bufs=2) as stp,
                  tc.tile_pool(name="ssp", bufs=2, space="PSUM") as ssp,
                  tc.tile_pool(name="bcp", bufs=2, space="PSUM") as bcp):
                # interleave pre/cur chunks: row h of stage B needs pre rows
                # h..h+16 (pre chunk ~1.2/batch) and cur rows h..h+3 (cur
                # chunk 1/batch); emit 6 pre chunks ahead, then ~3:2
                order = []
                pc, cc = 0, 0
                NPC, NCC = PRE_COLS // 480, CUR_COLS // 512
                while pc < NPC or cc < NCC:
                    if pc < NPC and (pc < 6 + (3 * cc) // 2 or cc >= NCC):
                        order.append((pre_n, pre_flat, 480, pc))
                        pc += 1
                    else:
                        order.append((cur_n, cur_flat, 512, cc))
                        cc += 1
                for tiles, flats, cw, j in order:
                    if True:
                        cs = slice(j * cw, (j + 1) * cw)
                        ss = ssp.tile([1, cw], f32, tag="ss", name="ss")
                        for c in range(2):
                            nc.sync.dma_start(tiles[c][:, cs], flats[c][:, cs])
                            sq = sqp.tile([128, cw], f16, tag="sq", name="sq")
                            nc.scalar.activation(sq[:, :], tiles[c][:, cs], AF.Square)
                            nc.tensor.matmul(ss[:, :], ones_col[:, :], sq[:, :],
                                             start=(c == 0), stop=(c == 1))
                        inv1 = stp.tile([1, cw], f16, tag="inv1", name="inv1")
                        nc.scalar.activation(inv1[:, :], ss[:, :],
                                             AF.Abs_reciprocal_sqrt)
                        bc = bcp.tile([128, cw], f32, tag="bc", name="bc")
                        nc.tensor.matmul(bc[:, :], ones_row[:, :], inv1[:, :],
                                         start=True, stop=True)
                        inv = sqp.tile([128, cw], f16, tag="inv", name="inv")
                        nc.scalar.activation(inv[:, :], bc[:, :], AF.Copy)
                        for c in range(2):
                            nc.gpsimd.tensor_tensor(
                                out=tiles[c][:, cs], in0=tiles[c][:, cs],
                                in1=inv[:, :], op=ALU.mult)

            # ---- Stage B: per-row gram, shear, compact; batched sort; place ----
            with (tc.tile_pool(name="stage", bufs=4) as stgp,
                  tc.tile_pool(name="cop", bufs=4) as cop,
                  tc.tile_pool(name="idxp", bufs=4) as idxp,
                  tc.tile_pool(name="xsp", bufs=3) as xsp,
                  tc.tile_pool(name="ysp", bufs=3) as ysp,
                  tc.tile_pool(name="dstp", bufs=3) as dstp,
                  tc.tile_pool(name="fop", bufs=3) as fop,
                  tc.tile_pool(name="cvp", bufs=3) as cvp,
                  tc.tile_pool(name="trp", bufs=3) as trp,
                  tc.tile_pool(name="gp", bufs=1, space="PSUM") as gp,
                  tc.tile_pool(name="tp", bufs=1, space="PSUM") as tp):
                def emit_placement(xs, hb):
                    for hr in range(RBATCH):
                        h = hb + hr
                        for t, out_d, tag in ((0, f_d, "f"), (1, b_d, "b")):
                            arr = hr * 2 + t
                            st = xs[:, arr * 256:(arr + 1) * 256]
                            # dest = j + 33*[v<0]; zero-pad vs true-zero delta
                            ng = dstp.tile([128, 256], f16, tag=f"ng{tag}", name=f"ng{tag}")
                            nc.gpsimd.tensor_scalar(
                                out=ng[:, :], in0=st,
                                scalar1=0.0, scalar2=float(D - 256),
                                op0=ALU.is_lt, op1=ALU.mult)
                            nc.gpsimd.tensor_tensor(
                                out=ng[:, :], in0=ng[:, :], in1=iota[:, :], op=ALU.add)
                            di = dstp.tile([128, 256], i16, tag=f"di{tag}", name=f"di{tag}")
                            nc.scalar.activation(di[:, :], ng[:, :], AF.Copy)
                            fo = fop.tile([128, 290], f16, tag=f"fo{tag}", name=f"fo{tag}")
                            nc.gpsimd.local_scatter(
                                fo[:, :], st, di[:, :],
                                channels=128, num_elems=290, num_idxs=256)
                            cv = cvp.tile([128, 256], f32, tag=f"cv{tag}", name=f"cv{tag}")
                            nc.scalar.activation(cv[:, :], fo[:, 0:256], AF.Copy)
                            pt = tp.tile([128, 256], f32, tag=f"pt{tag}", name=f"pt{tag}")
                            for half in range(2):
                                nc.tensor.transpose(
                                    pt[:, half * 128:(half + 1) * 128],
                                    cv[:, half * 128:(half + 1) * 128],
                                    ident[:, :])
                            tr = trp.tile([128, 256], f32, tag=f"tr{tag}", name=f"tr{tag}")
                            nc.scalar.activation(tr[:, :], pt[:, :], AF.Copy)
                            oview = out_d.rearrange("(cc p) hh ww -> p cc hh ww", cc=2)
                            nc.sync.dma_start(
                                oview[:, :, h, :],
                                tr[:, :].rearrange("p (cc ww) -> p cc ww", ww=W))

                pending = None
                for hb in range(0, HSLICE, RBATCH):
                    xs = xsp.tile([128, SORTW], f16, tag="xs", name="xs")
                    ys = ysp.tile([128, SORTW], f16, tag="ys", name="ys")
                    for hr in range(RBATCH):
                        h = hb + hr
                        sc = scr[h % 4]
                        stage = stgp.tile([128, STG], f16, tag="stage", name="stage")
                        for gi, (dy0, ndy) in enumerate(GRP):
                            g = gp.tile([128, ndy * WP], f32, tag=f"g{gi}", name=f"g{gi}")
                            for li in range(ndy):
                                dy = dy0 + li
                                for c in range(2):
                                    nc.tensor.matmul(
                                        g[:, li * WP:(li + 1) * WP],
                                        cur_n[c][:, h * W:(h + 1) * W],
                                        pre_n[c][:, (h + dy) * WP:(h + dy + 1) * WP],
                                        start=(c == 0), stop=(c == 1))
                            nc.scalar.activation(
                                stage[:, dy0 * WP:(dy0 + ndy) * WP], g[:, :], AF.Copy)
                        wview = sc[0:128 * (STG + 1)].rearrange("(p r) -> p r", r=STG + 1)
                        nc.sync.dma_start(wview[:, 0:STG], stage[:, :])
                        co = cop.tile([128, NIC], f16, tag="co", name="co")
                        rview = sc[:].rearrange("(p r) -> p r", r=STG + 2)
                        rview = rview[:, 0:STG].rearrange("p (a b) -> p a b", b=WP)
                        nc.sync.dma_start(co[:, 0:D], rview[:, :, 0:KW])
                        nc.gpsimd.memset(co[:, D:NIC], -3.0)

                        idxf = idxp.tile([128, NIC], i16, tag="idxf", name="idxf")
                        idxb = idxp.tile([128, NIC], i16, tag="idxb", name="idxb")
                        nc.sync.dma_start(idxf[:, :], idxf_d[h, :, :])
                        nc.sync.dma_start(idxb[:, :], idxb_d[h, :, :])
                        for t, idxt in ((0, idxf), (1, idxb)):
                            arr = hr * 2 + t
                            nc.gpsimd.local_scatter(
                                xs[:, arr * 256:(arr + 1) * 256], co[:, :], idxt[:, :],
                                channels=128, num_elems=256, num_idxs=NIC)

                    # previous batch's placement: after this batch's compaction
                    # scatters so Pool keeps the sorter fed first
                    if pending is not None:
                        emit_placement(*pending)

                    # ---- batched descending bitonic sort of 8 x 256 ----
                    cur_t, nxt_t = xs, ys
                    for kind, v in _bitonic_stages(256):
                        if kind == 'flip':
                            hh = v // 2
                            cv_ = cur_t[:, :].rearrange(
                                "p (A two hh) -> p A two hh", two=2, hh=hh)
                            nv_ = nxt_t[:, :].rearrange(
                                "p (A two hh) -> p A two hh", two=2, hh=hh)
                            lo_i, hi_i = cv_[:, :, 0, :], cv_[:, :, 1, ::-1]
                            lo_o, hi_o = nv_[:, :, 0, :], nv_[:, :, 1, ::-1]
                        else:
                            dd = v
                            cv_ = cur_t[:, :].rearrange(
                                "p (A two dd) -> p A two dd", two=2, dd=dd)
                            nv_ = nxt_t[:, :].rearrange(
                                "p (A two dd) -> p A two dd", two=2, dd=dd)
                            lo_i, hi_i = cv_[:, :, 0, :], cv_[:, :, 1, :]
                            lo_o, hi_o = nv_[:, :, 0, :], nv_[:, :, 1, :]
                        nc.vector.tensor_tensor(out=lo_o, in0=lo_i, in1=hi_i, op=ALU.max)
                        nc.vector.tensor_tensor(out=hi_o, in0=lo_i, in1=hi_i, op=ALU.min)
                        cur_t, nxt_t = nxt_t, cur_t
                    # 36 stages (even) -> result back in xs
                    pending = (xs, hb)
                emit_placement(*pending)
    nc.compile()
    return nc


def _mask_params(mask):
    """Sanity-check the mask against the fixed 256-slot sort arrays."""
    mp = np.pad(mask, ((0, 0), (0, 0), (K, K), (K, K)))[:, 0]
    n1 = np.zeros((B, H, W), np.int64)
    for dy in range(KW):
        for dx in range(KW):
            n1 += (mp[:, dy:dy + H, dx:dx + W] != 0)
    assert int(n1.max()) <= 256 and int((D - n1).max()) <= 256, (
        "mask window occupancy exceeds 256; fixed-width sort arrays too small")
    return ("v2",)


def _host_prep(pre, curr, mask, params=None):
    pre = np.asarray(pre, dtype=np.float32)
    curr = np.asarray(curr, dtype=np.float32)
    pre_pad = np.pad(pre, ((0, 0), (0, 0), (K, K), (K, K)), mode="reflect")
    mask_pad = np.pad(np.asarray(mask, np.float32), ((0, 0), (0, 0), (K, K), (K, K)))
    ins = []
    for k in range(N_CORES):
        b, hh = k // 2, k % 2
        h0 = hh * HSLICE
        mpk = mask_pad[b, 0, h0:h0 + HP, :]
        s0, s1 = mpk.strides
        m_unf = np.lib.stride_tricks.as_strided(
            mpk, (HSLICE, KW, W, KW), (s0, s0, s1, s1))
        m_unf = (m_unf.transpose(0, 2, 1, 3).reshape(HSLICE, W, D) != 0)
        # compaction scatter indices: dest rank among masked / unmasked
        csum1 = np.cumsum(m_unf, axis=-1, dtype=np.int32)
        rank1 = csum1 - m_unf  # exclusive cumsum
        d_ar = np.arange(D, dtype=np.int32)
        # rank among unmasked = d - (#masked in 0..d); valid at unmasked slots
        rank0 = d_ar[None, None, :] - csum1
        idx_f = np.where(m_unf, rank1, -1).astype(np.int16)
        idx_b = np.where(~m_unf, rank0, -1).astype(np.int16)
        pad_col = np.full((HSLICE, W, 1), -1, np.int16)
        idx_f = np.concatenate([idx_f, pad_col], axis=-1)
        idx_b = np.concatenate([idx_b, pad_col], axis=-1)
        ins.append({
            "pre_pad": np.ascontiguousarray(
                pre_pad[b, :, h0:h0 + HP, :]).astype(np.float16),
            "curr": np.ascontiguousarray(
                curr[b, :, h0:h0 + HSLICE, :]).astype(np.float16),
            "idx_f": np.ascontiguousarray(idx_f),
            "idx_b": np.ascontiguousarray(idx_b),
        })
    return ins


def kernel(pre, curr, mask, mode):
    from concourse.bass_utils import run_bass_kernel_spmd

    pre = np.asarray(pre, dtype=np.float32)
    curr = np.asarray(curr, dtype=np.float32)
    mask = np.asarray(mask, dtype=np.float32)
    assert int(np.asarray(mode)) == 0

    params = _mask_params(mask)
    if _CACHED.get("params") != params:
        _CACHED["nc"] = _build_nc()
        _CACHED["params"] = params
    nc = _CACHED["nc"]

    in_maps = _host_prep(pre, curr, mask, params)
    res = run_bass_kernel_spmd(nc, in_maps, core_ids=list(range(N_CORES)))
    f = np.zeros((B, TOPK, H, W), np.float32)
    bo = np.zeros((B, TOPK, H, W), np.float32)
    for k in range(N_CORES):
        bb, hh = k // 2, k % 2
        f[bb, :, hh * HSLICE:(hh + 1) * HSLICE, :] = res.results[k]["f_out"]
        bo[bb, :, hh * HSLICE:(hh + 1) * HSLICE, :] = res.results[k]["b_out"]
    return (f, bo)
